# revision 20
# baseline (speedup 1.0000x reference)
"""MetaNet (2-layer GNN message passing) on 8 Trainium2 NeuronCores.

v2: bf16 edge streams + weights (fp32 PSUM accumulation), int8 one-hot
builds, row-tiled concurrent matmuls for the m-MLP first layer, per-tile
U = x1 @ W_row folding for layer 2's x1[row] term (via the selN one-hot),
fused DVE bias+relu from PSUM, split AllGather overlapped with layer 1.

Sharding: edges sorted by destination node; nodes (and their incoming
edges) partitioned into 8 contiguous ranges, one per core. scatter_mean
is a per-node-tile segment-sum done as a one-hot (is_equal) matmul into a
persistent PSUM accumulator; x1 is AllGathered between the two layers.
"""

import sys

sys.path.insert(0, "/opt/trn_rl_repo")

import numpy as np
import ml_dtypes

import concourse.bass as bass
import concourse.mybir as mybir
import concourse.tile as tile
from concourse.tile import add_dep_helper
from concourse.bass_utils import run_bass_kernel_spmd

F32 = mybir.dt.float32
BF16 = mybir.dt.bfloat16
I32 = mybir.dt.int32
I8 = mybir.dt.int8
P = 128
CW = 512  # edges per full chunk
NCORES = 8
ACT = mybir.ActivationFunctionType
SPLIT_WAITS = True
BF = ml_dtypes.bfloat16


def _split_multi_waits(nc):
    # This container's walrus build accepts only ONE sync-wait command per
    # instruction. Hoist extra waits onto same-engine NOPs placed directly
    # before the instruction (sequencers run in order, so semantics match).
    n = 0
    for bb in nc.main_func.blocks:
        new_insts = []
        for ins in bb.instructions:
            si = getattr(ins, "sync_info", None)
            if si is not None and si.on_wait and len(si.on_wait) > 1:
                waits = list(si.on_wait)
                for w in waits[:-1]:
                    nop = mybir.InstNoOp(
                        name=f"wsplit_{n}",
                        engine=ins.engine,
                        bass_nofuse=True,
                        sync_info=mybir.SyncInfo(on_wait=[w], on_update=[]),
                    )
                    n += 1
                    new_insts.append(nop)
                si.on_wait = [waits[-1]]
            new_insts.append(ins)
        bb.instructions[:] = new_insts
    return n


def _host_prep(x, edge_attr, edge_index):
    N = x.shape[0]
    npc = ((N + NCORES - 1) // NCORES + P - 1) // P * P  # nodes/core, mult of 128
    NT = npc // P
    npad = npc * NCORES

    row = edge_index[0].astype(np.int64)
    col = edge_index[1].astype(np.int64)

    order = np.argsort(row, kind="stable")
    row_s, col_s = row[order], col[order]
    core_of = row_s // npc
    ltile = (row_s % npc) // P

    cnt_ct = np.zeros((NCORES, NT), np.int64)
    np.add.at(cnt_ct, (core_of, ltile), 1)
    k128_u = ((cnt_ct + P - 1) // P).max(axis=0)  # uniform subchunks per tile

    chunk_widths = []
    for t in range(NT):
        k = int(k128_u[t])
        full, rem = divmod(k, CW // P)
        chunk_widths.append([CW] * full + ([P * rem] if rem else []))
    E_pad = int(k128_u.sum()) * P

    rowrel = np.full((NCORES, E_pad), -1, np.int8)
    rowglob = np.zeros((NCORES, E_pad), np.int64)
    colg = np.zeros((NCORES, E_pad), np.int64)
    ea_perm = np.zeros((NCORES, E_pad), np.int64)
    ea_valid = np.zeros((NCORES, E_pad), bool)
    tstart = np.concatenate([[0], np.cumsum(k128_u) * P])[:-1]

    for c in range(NCORES):
        idx_c = np.nonzero(core_of == c)[0]
        lt_c = ltile[idx_c]
        ord_lt = np.argsort(lt_c, kind="stable")
        idx_c = idx_c[ord_lt]
        lt_sorted = lt_c[ord_lt]
        starts = np.searchsorted(lt_sorted, np.arange(NT))
        ends = np.searchsorted(lt_sorted, np.arange(NT), side="right")
        for t in range(NT):
            m = idx_c[starts[t]:ends[t]]
            n = len(m)
            if n == 0:
                continue
            o = int(tstart[t])
            rowrel[c, o:o + n] = (row_s[m] % P).astype(np.int8)
            rowglob[c, o:o + n] = row_s[m]
            colg[c, o:o + n] = col_s[m]
            ea_perm[c, o:o + n] = order[m]
            ea_valid[c, o:o + n] = True

    FE = edge_attr.shape[1]
    Fx = x.shape[1]
    # layer-1 edge stream, feature-major: [xrow(0:Fx); ea(Fx:Fx+FE); xcol]
    ein1 = np.zeros((NCORES, 2 * Fx + FE, E_pad), BF)
    xb = x.astype(BF)
    eab = edge_attr.astype(BF)
    for c in range(NCORES):
        v = ea_valid[c]
        ein1[c][:Fx, v] = xb[rowglob[c][v]].T
        ein1[c][Fx:Fx + FE][:, v] = eab[ea_perm[c][v]].T
        ein1[c][Fx + FE:] = xb[colg[c]].T

    # chunk metadata + packed per-chunk row/col index tiles [NCHUNK, 128, 4]
    chunk_meta = []  # (tile, ebase, W, is_first, is_last, chunk_id)
    ci = 0
    base = 0
    for t in range(NT):
        ws = chunk_widths[t]
        for j, w in enumerate(ws):
            chunk_meta.append((t, base, w, j == 0, j == len(ws) - 1, ci))
            ci += 1
            base += w
    NCHUNK = ci
    assert base == E_pad

    rowp4 = np.full((NCORES, max(NCHUNK, 1), P, 4), -1, np.int8)
    colp4 = np.zeros((NCORES, max(NCHUNK, 1), P, 4), np.int32)
    for (t, ebase, w, _f, _l, cid) in chunk_meta:
        r = w // P
        for c in range(NCORES):
            rowp4[c, cid, :, :r] = rowrel[c, ebase:ebase + w].reshape(r, P).T
            colp4[c, cid, :, :r] = colg[c, ebase:ebase + w].reshape(r, P).T

    cnt = np.zeros(npad, np.int64)
    np.add.at(cnt, row, 1)
    inv = np.where(cnt > 0, 1.0 / np.maximum(cnt, 1), 0.0).astype(np.float32)
    msk = (cnt > 0).astype(np.float32)
    invP = inv.reshape(NCORES, NT, P).transpose(0, 2, 1).copy()
    mskP = msk.reshape(NCORES, NT, P).transpose(0, 2, 1).copy()

    x_own = np.zeros((npad, Fx), np.float32)
    x_own[:N] = x

    rowp4b = rowp4.transpose(0, 2, 1, 3).reshape(NCORES, P, -1)
    colp4b = colp4.transpose(0, 2, 1, 3).reshape(NCORES, P, -1)
    return dict(N=N, npc=npc, NT=NT, npad=npad, NCHUNK=NCHUNK, E_pad=E_pad,
                chunk_meta=chunk_meta, rowrel=rowrel, rowp4b=rowp4b,
                colp4b=colp4b, ein1=ein1, invP=invP, mskP=mskP,
                x_own=x_own.reshape(NCORES, npc, Fx))


def kernel(x, edge_attr, edge_index, **wts):
    x = np.asarray(x, np.float32)
    edge_attr = np.asarray(edge_attr, np.float32)
    edge_index = np.asarray(edge_index)
    wts = {k: np.asarray(v, np.float32) for k, v in wts.items()}
    import os
    return _run(x, edge_attr, edge_index, wts,
                trace=os.environ.get("BASS_KERNEL_TRACE", "0") == "1")


def _run(x, edge_attr, edge_index, wts, trace=False, build_only=False):
    import os
    STAGE = int(os.environ.get("V2_STAGE", "4"))
    SUB = int(os.environ.get("V2_SUB", "9"))
    pre = _host_prep(x, edge_attr, edge_index)
    F = x.shape[1]          # 32
    H = wts["e1_w2"].shape[1]  # 64
    FE = edge_attr.shape[1]  # 32
    npc, NT, NCHUNK, E_pad = pre["npc"], pre["NT"], pre["NCHUNK"], pre["E_pad"]

    # bias folding: e*_b2 folded into downstream first-layer biases (the ea
    # streams are stored pre-bias)
    b_hm1 = (wts["n1a_b1"] + wts["e1_b2"] @ wts["n1a_w1"][F:]).reshape(H, 1)
    b_e2 = (wts["e2_b1"] + wts["e1_b2"] @ wts["e2_w1"][2 * H:]).reshape(H, 1)
    b_hm2 = (wts["n2a_b1"] + wts["e2_b2"] @ wts["n2a_w1"][H:]).reshape(H, 1)

    def bf(a):
        return np.ascontiguousarray(a).astype(BF)

    n1a_wq = np.zeros((P, H), BF)
    n1a_wq[0:H] = bf(wts["n1a_w1"][F:])        # ea part
    n1a_wq[H:H + F] = bf(wts["n1a_w1"][:F])    # xcol part
    n2a_wq = np.zeros((P, H), BF)
    n2a_wq[0:H] = bf(wts["n2a_w1"][H:])        # ea part
    n2a_wq[H:2 * H] = bf(wts["n2a_w1"][:H])    # xcol part
    e2_w1bc = np.zeros((P, H), BF)
    e2_w1bc[0:H] = bf(wts["e2_w1"][2 * H:])    # ea part
    e2_w1bc[H:2 * H] = bf(wts["e2_w1"][H:2 * H])  # xcol part

    # ein1 feature order: [xrow; ea; xcol]
    e1_w1r = np.concatenate([wts["e1_w1"][:F], wts["e1_w1"][2 * F:],
                             wts["e1_w1"][F:2 * F]])

    cb = dict(  # bf16 weight consts
        e1_w1r=bf(e1_w1r), e1_w2=bf(wts["e1_w2"]),
        n1a_wq=n1a_wq, n1a_w2=bf(wts["n1a_w2"]),
        n1b_wq=bf(np.concatenate([wts["n1b_w1"][F:], wts["n1b_w1"][:F]])),
        n1b_w2=bf(wts["n1b_w2"]),
        e2_w1u=bf(wts["e2_w1"][:H]), e2_w1bc=e2_w1bc, e2_w2=bf(wts["e2_w2"]),
        n2a_wq=n2a_wq, n2a_w2=bf(wts["n2a_w2"]),
        n2b_wq=bf(np.concatenate([wts["n2b_w1"][H:], wts["n2b_w1"][:H]])),
        n2b_w2=np.concatenate([bf(wts["n2b_w2"]), np.zeros((H, 1), BF)], axis=1),
        iotaF4=np.tile(np.arange(P, dtype=np.int8)[None, :], (P, 4)),
    )
    cf = dict(  # f32 consts
        e1_b1=wts["e1_b1"].reshape(H, 1), b_hm1=b_hm1,
        n1b_b1=wts["n1b_b1"].reshape(H, 1), n1b_b2=wts["n1b_b2"].reshape(H, 1),
        b_e2=b_e2, b_hm2=b_hm2, n2b_b1=wts["n2b_b1"].reshape(H, 1),
        b2bc1=np.tile(wts["n1a_b2"][None, :], (P, 1)).astype(np.float32),
        b2bc2=np.tile(wts["n2a_b2"][None, :], (P, 1)).astype(np.float32),
        identF=np.eye(P, dtype=np.float32),
    )
    ci8 = dict(iotaP8=np.arange(P, dtype=np.int8).reshape(P, 1))
    n2b_b2_val = float(wts["n2b_b2"].reshape(-1)[0])

    nc = bass.Bass(num_swdge_queues=4)

    dp = {}
    for k, v in cb.items():
        dp[k] = nc.declare_dram_parameter(k, list(v.shape),
                                          I8 if v.dtype == np.int8 else BF16,
                                          isOutput=False)
    for k, v in cf.items():
        dp[k] = nc.declare_dram_parameter(k, list(v.shape), F32, isOutput=False)
    for k, v in ci8.items():
        dp[k] = nc.declare_dram_parameter(k, list(v.shape), I8, isOutput=False)

    xo_d = nc.declare_dram_parameter("x_own", [npc, F], F32, isOutput=False)
    ein1_d = nc.declare_dram_parameter("ein1", [2 * F + FE, E_pad], BF16,
                                       isOutput=False)
    rowf_d = nc.declare_dram_parameter("rowflat", [E_pad], I8, isOutput=False)
    NC4 = max(NCHUNK, 1) * 4
    rowp4_d = nc.declare_dram_parameter("rowp4b", [P, NC4], I8, isOutput=False)
    colp4_d = nc.declare_dram_parameter("colp4b", [P, NC4], I32, isOutput=False)
    inv_d = nc.declare_dram_parameter("invP", [P, NT], F32, isOutput=False)
    msk_d = nc.declare_dram_parameter("mskP", [P, NT], F32, isOutput=False)
    x2_d = nc.declare_dram_parameter("x2", [npc, 4], F32, isOutput=True)

    cm = pre["chunk_meta"]

    with tile.TileContext(nc) as tc:
        with (
            tc.tile_pool(name="cst", bufs=1) as cst,
            tc.tile_pool(name="sb", bufs=2) as sb,
            tc.tile_pool(name="sb3", bufs=3) as sb3,
            tc.tile_pool(name="ps_tc", bufs=1, space="PSUM") as ps_tc,
            tc.tile_pool(name="ps_h1", bufs=2, space="PSUM") as ps_h1,
            tc.tile_pool(name="ps_ea", bufs=1, space="PSUM") as ps_ea,
            tc.tile_pool(name="ps_hm", bufs=1, space="PSUM") as ps_hm,
            tc.tile_pool(name="ps_me", bufs=1, space="PSUM") as ps_me,
            tc.tile_pool(name="ps_ag", bufs=2, space="PSUM") as ps_ag,
            tc.tile_pool(name="dram", bufs=1, space="DRAM") as dram,
        ):
            ct = {}
            for k, v in {**cb, **cf, **ci8}.items():
                dt_ = (I8 if v.dtype == np.int8 else
                       BF16 if v.dtype == BF else F32)
                t_ = cst.tile(list(v.shape), dt_, name=f"c_{k}")
                nc.sync.dma_start(out=t_[:], in_=dp[k][:])
                ct[k] = t_
            r4all = cst.tile([P, NC4], I8, name="c_r4")
            nc.sync.dma_start(out=r4all[:], in_=rowp4_d[:])
            c4all = cst.tile([P, NC4], I32, name="c_c4")
            nc.sync.dma_start(out=c4all[:], in_=colp4_d[:])
            invT = cst.tile([P, NT], F32, name="c_inv")
            nc.sync.dma_start(out=invT[:], in_=inv_d[:])
            mskT = cst.tile([P, NT], F32, name="c_msk")
            nc.sync.dma_start(out=mskT[:], in_=msk_d[:])

            ea1T_d = dram.tile([H, E_pad], BF16, name="ea1T")
            x1own_d = dram.tile([npc, H], F32, name="x1own")
            x1full_d = dram.tile([NCORES * npc, H], F32, name="x1full",
                                 addr_space="Shared")

            iF = ct["identF"]

            def relu_dve(out_ap, in_ap, bias_ap):
                nc.vector.tensor_scalar(
                    out=out_ap, in0=in_ap, scalar1=bias_ap, scalar2=0.0,
                    op0=mybir.AluOpType.add, op1=mybir.AluOpType.max)

            def scatter_tail(t, W, R, cid, isf, isl, hm_sb, mw2, agg_ps):
                # second m-MLP layer (edge-major, fused transpose) + one-hot
                # scatter-accumulate into the tile's PSUM agg
                me_ps = ps_me.tile([P, 4 * H], F32, tag="me")
                for k in range(R):
                    nc.tensor.matmul(me_ps[:, k * H:(k + 1) * H],
                                     lhsT=hm_sb[:, k * P:(k + 1) * P],
                                     rhs=ct[mw2][:], start=True, stop=True)
                me_sb = sb.tile([P, 4 * H], BF16, tag="mesb")
                nc.vector.tensor_copy(me_sb[:, :R * H], me_ps[:, :R * H])
                selT = sb.tile([P, 4, P], BF16, tag="selT")
                nc.vector.tensor_tensor(
                    out=selT[:, :R, :],
                    in0=r4all[:, cid * 4:cid * 4 + R].to_broadcast([P, R, P]),
                    in1=ct["iotaF4"][:, :R * P].rearrange("p (a b) -> p a b", a=R),
                    op=mybir.AluOpType.is_equal)
                for k in range(R):
                    nc.tensor.matmul(agg_ps[:], lhsT=selT[:, k, :],
                                     rhs=me_sb[:, k * H:(k + 1) * H],
                                     start=(isf and k == 0),
                                     stop=(isl and k == R - 1),
                                     skip_group_check=True)

            def node_tail(t, xw, x_tT_ps, agg_ps, has_edges, b2bc, nwq, nb1,
                          out_hook, nin, invT=None, mskT=None):
                # mean + bias-mask + node MLP; nin[H:H+xw] already holds x^T
                agg_sb = sb.tile([P, H], F32, tag="aggsb")
                if has_edges:
                    nc.vector.tensor_scalar(
                        out=agg_sb[:], in0=agg_ps[:], scalar1=invT[:, t:t + 1],
                        scalar2=None, op0=mybir.AluOpType.mult)
                else:
                    nc.vector.memset(agg_sb[:], 0.0)
                b2m = sb.tile([P, H], F32, tag="b2m")
                nc.vector.tensor_scalar(
                    out=b2m[:], in0=ct[b2bc][:], scalar1=mskT[:, t:t + 1],
                    scalar2=None, op0=mybir.AluOpType.mult)
                nc.vector.tensor_tensor(out=agg_sb[:], in0=agg_sb[:],
                                        in1=b2m[:], op=mybir.AluOpType.add)
                aT_ps = ps_tc.tile([H, P], F32, tag="tc")
                nc.tensor.transpose(out=aT_ps[:], in_=agg_sb[:], identity=iF[:])
                nc.scalar.activation(nin[0:H, :], aT_ps[:], ACT.Copy)
                hn_ps = ps_hm.tile([H, P], F32, tag="hm")
                nc.tensor.matmul(hn_ps[:], lhsT=ct[nwq][:H + xw, :],
                                 rhs=nin[:H + xw, :], start=True, stop=True)
                hn = sb.tile([H, P], BF16, tag="hn")
                relu_dve(hn[:], hn_ps[:], ct[nb1][:, :1])
                out_hook(t, hn)

            # ---------------- layer 1 ----------------
            ag_map = {NT - 1: (0, npc)}

            def do_allgather(lo, hi):
                nc.gpsimd.collective_compute(
                    "AllGather", mybir.AluOpType.bypass,
                    replica_groups=[list(range(NCORES))],
                    ins=[x1own_d[lo:hi].opt()],
                    outs=[x1full_d.rearrange("(c n) h -> c n h", c=NCORES)[:, lo:hi]
                          .opt()])

            def out1(t, hn):
                x1T_ps = ps_ea.tile([H, P], F32, tag="ea")
                nc.tensor.matmul(x1T_ps[:], lhsT=ct["n1b_w2"][:], rhs=hn[:],
                                 start=True, stop=True)
                x1T = sb.tile([H, P], F32, tag="x1T")
                nc.scalar.activation(x1T[:], x1T_ps[:], ACT.Relu,
                                     bias=ct["n1b_b2"][:, :1])
                x1_ps = ps_me.tile([P, H], F32, tag="me")
                nc.tensor.transpose(out=x1_ps[:], in_=x1T[:],
                                    identity=iF[:H, :H])
                x1sb = sb.tile([P, H], F32, tag="x1sb")
                nc.vector.tensor_copy(x1sb[:], x1_ps[:])
                nc.sync.dma_start(out=x1own_d[t * P:(t + 1) * P, :], in_=x1sb[:])
                if t in ag_map:
                    do_allgather(*ag_map[t])

            for t in range(NT):
                chunks = [c for c in cm if c[0] == t]
                x_t = sb.tile([P, F], F32, tag="x_t")
                nc.sync.dma_start(out=x_t[:], in_=xo_d[t * P:(t + 1) * P, :])
                xT_ps = ps_tc.tile([F, P], F32, tag="tc")
                nc.tensor.transpose(out=xT_ps[:], in_=x_t[:], identity=iF[:])
                nin = sb.tile([H + F, P], BF16, tag="nin1")
                nc.scalar.activation(nin[H:H + F, :], xT_ps[:], ACT.Copy)
                agg_ps = ps_ag.tile([P, H], F32, tag="agg")
                for (tt, ebase, W, isf, isl, cid) in chunks:
                    R = W // P
                    rhsF = sb.tile([2 * F + FE, CW], BF16, tag="rhsF")
                    nc.sync.dma_start(out=rhsF[:, :W],
                                      in_=ein1_d[:, ebase:ebase + W])
                    h1_ps = ps_h1.tile([H, CW], F32, tag="h1")
                    nc.tensor.matmul(h1_ps[:, :W], lhsT=ct["e1_w1r"][:],
                                     rhs=rhsF[:, :W], start=True, stop=True)
                    h1r = sb.tile([H, CW], BF16, tag="h1r")
                    relu_dve(h1r[:, :W], h1_ps[:, :W], ct["e1_b1"][:, :1])
                    ea_ps = ps_ea.tile([H, CW], F32, tag="ea")
                    nc.tensor.matmul(ea_ps[:, :W], lhsT=ct["e1_w2"][:],
                                     rhs=h1r[:, :W], start=True, stop=True)
                    m_in = sb.tile([H + F, CW], BF16, tag="m_in1")
                    nc.scalar.activation(m_in[0:H, :W], ea_ps[:, :W], ACT.Copy)
                    nc.scalar.dma_start(out=ea1T_d[:, ebase:ebase + W],
                                        in_=m_in[0:H, :W])
                    nc.vector.tensor_copy(m_in[H:H + F, :W],
                                          rhsF[F + FE:, :W])
                    hm_ps = ps_hm.tile([H, CW], F32, tag="hm")
                    nc.tensor.matmul(hm_ps[:, :W], lhsT=ct["n1a_wq"][0:H + F, :],
                                     rhs=m_in[:, :W], start=True, stop=True)
                    hm_sb = sb.tile([H, CW], BF16, tag="hm_sb")
                    relu_dve(hm_sb[:, :W], hm_ps[:, :W], ct["b_hm1"][:, :1])
                    scatter_tail(t, W, R, cid, isf, isl, hm_sb, "n1a_w2",
                                 agg_ps)
                node_tail(t, F, xT_ps, agg_ps, bool(chunks), "b2bc1",
                          "n1b_wq", "n1b_b1", out1, nin, invT, mskT)

            # ---------------- layer 2 ----------------
            def out2(t, hn):
                x2_ps = ps_me.tile([P, 4], F32, tag="me")
                nc.tensor.matmul(x2_ps[:, :2], lhsT=hn[:],
                                 rhs=ct["n2b_w2"][:], start=True, stop=True)
                x2sb = sb.tile([P, 4], F32, tag="x2sb")
                nc.scalar.activation(x2sb[:, :1], x2_ps[:, :1], ACT.Copy,
                                     bias=n2b_b2_val)
                nc.sync.dma_start(out=x2_d[t * P:(t + 1) * P, :1], in_=x2sb[:, :1])

            for t in range(NT if STAGE >= 2 else 0):
                chunks = [c for c in cm if c[0] == t]
                x_t = sb.tile([P, H], F32, tag="x_t")
                nc.sync.dma_start(out=x_t[:], in_=x1own_d[t * P:(t + 1) * P, :])
                xT_ps = ps_tc.tile([H, P], F32, tag="tc")
                nc.tensor.transpose(out=xT_ps[:], in_=x_t[:], identity=iF[:])
                nin = sb.tile([2 * H, P], BF16, tag="nin2")
                nc.scalar.activation(nin[H:2 * H, :], xT_ps[:], ACT.Copy)
                # U^T = x1_tile @ e2_w1[0:H] (the x1[row] part of the edge MLP)
                x1T_sb = sb.tile([H, P], BF16, tag="x1Tsb")
                nc.scalar.activation(x1T_sb[:], xT_ps[:], ACT.Copy)
                ut_ps = ps_h1.tile([P, H], F32, tag="h1")
                nc.tensor.matmul(ut_ps[:], lhsT=x1T_sb[:],
                                 rhs=ct["e2_w1u"][:], start=True, stop=True)
                ut_sb = sb.tile([P, H], BF16, tag="ut")
                nc.vector.tensor_copy(ut_sb[:], ut_ps[:])
                agg_ps = ps_ag.tile([P, H], F32, tag="agg")
                if SUB < 2:
                    chunks = []
                for (tt, ebase, W, isf, isl, cid) in chunks:
                    R = W // P
                    selN = sb.tile([P, CW], BF16, tag="selN")
                    if STAGE >= 3:
                        rb = sb.tile([P, CW], I8, tag="rb")
                        nc.scalar.dma_start(
                            out=rb[:, :W],
                            in_=rowf_d[None, ebase:ebase + W].to_broadcast([P, W]))
                        nc.vector.tensor_tensor(
                            out=selN[:, :W],
                            in0=ct["iotaP8"][:, 0:1].to_broadcast([P, W]),
                            in1=rb[:, :W], op=mybir.AluOpType.is_equal)
                    else:
                        nc.vector.memset(selN[:, :W], 0.0)
                    bc = sb.tile([P, CW], BF16, tag="bc")
                    if SUB < 2:
                        nc.vector.memset(bc[0:H, :W], 0.0)
                    else:
                        nc.sync.dma_start(out=bc[0:H, :W],
                                          in_=ea1T_d[:, ebase:ebase + W])
                    if STAGE >= 4:
                        tc_ps = ps_tc.tile([H, CW], F32, tag="tc")
                        for k in range(R):
                            g = sb3.tile([P, H], F32, tag=f"g{k}")
                            nc.gpsimd.indirect_dma_start(
                                out=g[:], out_offset=None, in_=x1full_d[:],
                                in_offset=bass.IndirectOffsetOnAxis(
                                    ap=c4all[:, cid * 4 + k:cid * 4 + k + 1],
                                    axis=0))
                            nc.tensor.transpose(
                                out=tc_ps[:, k * P:(k + 1) * P],
                                in_=g[:], identity=iF[:])
                        nc.scalar.activation(bc[H:2 * H, :W], tc_ps[:, :W],
                                             ACT.Copy)
                    else:
                        nc.vector.memset(bc[H:2 * H, :W], 0.0)
                    h1r = sb.tile([H, CW], BF16, tag="h1r")
                    if SUB >= 3:
                        h1_ps = ps_h1.tile([H, CW], F32, tag="h1")
                        i1 = nc.tensor.matmul(h1_ps[:, :W], lhsT=ct["e2_w1bc"][:],
                                              rhs=bc[:, :W], start=True, stop=False,
                                              skip_group_check=True)
                        i2 = nc.tensor.matmul(h1_ps[:, :W], lhsT=ut_sb[:],
                                              rhs=selN[:, :W], start=False, stop=True,
                                              skip_group_check=True)
                        add_dep_helper(i2.ins, i1.ins, sync=False,
                                       reason="h1 accumulate order")
                        relu_dve(h1r[:, :W], h1_ps[:, :W], ct["b_e2"][:, :1])
                    else:
                        nc.vector.memset(h1r[:, :W], 0.0)
                    m_in = sb.tile([2 * H, CW], BF16, tag="m_in2")
                    if SUB >= 4:
                        ea_ps = ps_ea.tile([H, CW], F32, tag="ea")
                        nc.tensor.matmul(ea_ps[:, :W], lhsT=ct["e2_w2"][:],
                                         rhs=h1r[:, :W], start=True, stop=True)
                        nc.scalar.activation(m_in[0:H, :W], ea_ps[:, :W],
                                             ACT.Copy)
                    else:
                        nc.vector.memset(m_in[0:H, :W], 0.0)
                    nc.vector.tensor_copy(m_in[H:2 * H, :W], bc[H:2 * H, :W])
                    hm_sb = sb.tile([H, CW], BF16, tag="hm_sb")
                    if SUB >= 5:
                        hm_ps = ps_hm.tile([H, CW], F32, tag="hm")
                        nc.tensor.matmul(hm_ps[:, :W], lhsT=ct["n2a_wq"][:],
                                         rhs=m_in[:, :W], start=True, stop=True)
                        relu_dve(hm_sb[:, :W], hm_ps[:, :W], ct["b_hm2"][:, :1])
                    else:
                        nc.vector.memset(hm_sb[:, :W], 0.0)
                    if SUB >= 6:
                        scatter_tail(t, W, R, cid, isf, isl, hm_sb, "n2a_w2",
                                     agg_ps)
                node_tail(t, H, xT_ps, agg_ps, bool(chunks) and SUB >= 6, "b2bc2",
                          "n2b_wq", "n2b_b1", out2, nin, invT, mskT)

    if SPLIT_WAITS:
        _split_multi_waits(nc)

    in_maps = []
    for c in range(NCORES):
        m = dict(cb)
        m.update(cf)
        m.update(ci8)
        m["x_own"] = pre["x_own"][c]
        m["ein1"] = pre["ein1"][c]
        m["rowflat"] = pre["rowrel"][c]
        m["rowp4b"] = pre["rowp4b"][c]
        m["colp4b"] = pre["colp4b"][c]
        m["invP"] = pre["invP"][c]
        m["mskP"] = pre["mskP"][c]
        in_maps.append(m)

    kernel.last_nc = nc
    kernel.last_in_maps = in_maps
    if build_only:
        return pre
    r = run_bass_kernel_spmd(nc, in_maps, list(range(NCORES)), trace=trace)
    kernel.last_results = r
    out = np.concatenate([r.results[c]["x2"][:, :1] for c in range(NCORES)], axis=0)
    return out[:pre["N"]].astype(np.float32)


# revision 22
# speedup vs baseline: 1.2113x; 1.2113x over previous
"""MetaNet (2-layer GNN message passing) on 8 Trainium2 NeuronCores.

v3: bf16 edge streams + weights (fp32 PSUM accumulation), bf16 one-hot
builds hitting the DVE 2x/4x fast paths, per-tile U = x1 @ W_row folding
for layer 2's x1[row] term (applied via the selN one-hot matmul),
DVE/ACT-balanced elementwise ops, chunk-pair batched DMA loads.

Sharding: edges sorted by destination node; nodes (and their incoming
edges) partitioned into 8 contiguous ranges, one per core. scatter_mean
is a per-node-tile segment-sum done as a one-hot (is_equal) matmul into a
persistent PSUM accumulator; x1 is AllGathered between the two layers.
"""

import sys

sys.path.insert(0, "/opt/trn_rl_repo")

import numpy as np
import ml_dtypes

import concourse.bass as bass
import concourse.mybir as mybir
import concourse.tile as tile
from concourse.tile import add_dep_helper
from concourse.bass_utils import run_bass_kernel_spmd

F32 = mybir.dt.float32
BF16 = mybir.dt.bfloat16
I32 = mybir.dt.int32
P = 128
CW = 512  # edges per full chunk
NCORES = 8
ACT = mybir.ActivationFunctionType
SPLIT_WAITS = True
BF = ml_dtypes.bfloat16


def _split_multi_waits(nc):
    # This container's walrus build accepts only ONE sync-wait command per
    # instruction. Hoist extra waits onto same-engine NOPs placed directly
    # before the instruction (sequencers run in order, so semantics match).
    n = 0
    for bb in nc.main_func.blocks:
        new_insts = []
        for ins in bb.instructions:
            si = getattr(ins, "sync_info", None)
            if si is not None and si.on_wait and len(si.on_wait) > 1:
                waits = list(si.on_wait)
                for w in waits[:-1]:
                    nop = mybir.InstNoOp(
                        name=f"wsplit_{n}",
                        engine=ins.engine,
                        bass_nofuse=True,
                        sync_info=mybir.SyncInfo(on_wait=[w], on_update=[]),
                    )
                    n += 1
                    new_insts.append(nop)
                si.on_wait = [waits[-1]]
            new_insts.append(ins)
        bb.instructions[:] = new_insts
    return n


def _host_prep(x, edge_attr, edge_index):
    N = x.shape[0]
    npc = ((N + NCORES - 1) // NCORES + P - 1) // P * P  # nodes/core, mult of 128
    NT = npc // P
    npad = npc * NCORES

    row = edge_index[0].astype(np.int64)
    col = edge_index[1].astype(np.int64)

    order = np.argsort(row, kind="stable")
    row_s, col_s = row[order], col[order]
    core_of = row_s // npc
    ltile = (row_s % npc) // P

    cnt_ct = np.zeros((NCORES, NT), np.int64)
    np.add.at(cnt_ct, (core_of, ltile), 1)
    k128_u = ((cnt_ct + P - 1) // P).max(axis=0)  # uniform subchunks per tile

    chunk_widths = []
    for t in range(NT):
        k = int(k128_u[t])
        full, rem = divmod(k, CW // P)
        chunk_widths.append([CW] * full + ([P * rem] if rem else []))
    E_pad = int(k128_u.sum()) * P

    rowrel = np.full((NCORES, E_pad), -1, np.int16)
    rowglob = np.zeros((NCORES, E_pad), np.int64)
    colg = np.zeros((NCORES, E_pad), np.int64)
    ea_perm = np.zeros((NCORES, E_pad), np.int64)
    ea_valid = np.zeros((NCORES, E_pad), bool)
    tstart = np.concatenate([[0], np.cumsum(k128_u) * P])[:-1]

    for c in range(NCORES):
        idx_c = np.nonzero(core_of == c)[0]
        lt_c = ltile[idx_c]
        ord_lt = np.argsort(lt_c, kind="stable")
        idx_c = idx_c[ord_lt]
        lt_sorted = lt_c[ord_lt]
        starts = np.searchsorted(lt_sorted, np.arange(NT))
        ends = np.searchsorted(lt_sorted, np.arange(NT), side="right")
        for t in range(NT):
            m = idx_c[starts[t]:ends[t]]
            n = len(m)
            if n == 0:
                continue
            o = int(tstart[t])
            rowrel[c, o:o + n] = (row_s[m] % P).astype(np.int16)
            rowglob[c, o:o + n] = row_s[m]
            colg[c, o:o + n] = col_s[m]
            ea_perm[c, o:o + n] = order[m]
            ea_valid[c, o:o + n] = True

    FE = edge_attr.shape[1]
    Fx = x.shape[1]
    # layer-1 edge stream, feature-major: [xrow(0:Fx); ea(Fx:Fx+FE); xcol]
    ein1 = np.zeros((NCORES, 2 * Fx + FE, E_pad), BF)
    xb = x.astype(BF)
    eab = edge_attr.astype(BF)
    for c in range(NCORES):
        v = ea_valid[c]
        ein1[c][:Fx, v] = xb[rowglob[c][v]].T
        ein1[c][Fx:Fx + FE][:, v] = eab[ea_perm[c][v]].T
        ein1[c][Fx + FE:] = xb[colg[c]].T

    # chunk metadata + packed per-chunk row/col index tiles [NCHUNK, 128, 4]
    chunk_meta = []  # (tile, ebase, W, is_first, is_last, chunk_id)
    ci = 0
    base = 0
    for t in range(NT):
        ws = chunk_widths[t]
        for j, w in enumerate(ws):
            chunk_meta.append((t, base, w, j == 0, j == len(ws) - 1, ci))
            ci += 1
            base += w
    NCHUNK = ci
    assert base == E_pad

    rowp4 = np.full((NCORES, max(NCHUNK, 1), P, 4), -1, np.int16)
    colp4 = np.zeros((NCORES, max(NCHUNK, 1), P, 4), np.int32)
    for (t, ebase, w, _f, _l, cid) in chunk_meta:
        r = w // P
        for c in range(NCORES):
            rowp4[c, cid, :, :r] = rowrel[c, ebase:ebase + w].reshape(r, P).T
            colp4[c, cid, :, :r] = colg[c, ebase:ebase + w].reshape(r, P).T

    cnt = np.zeros(npad, np.int64)
    np.add.at(cnt, row, 1)
    inv = np.where(cnt > 0, 1.0 / np.maximum(cnt, 1), 0.0).astype(np.float32)
    msk = (cnt > 0).astype(np.float32)
    invP = inv.reshape(NCORES, NT, P).transpose(0, 2, 1).copy()
    mskP = msk.reshape(NCORES, NT, P).transpose(0, 2, 1).copy()

    x_own = np.zeros((npad, Fx), np.float32)
    x_own[:N] = x

    rowp4b = rowp4.transpose(0, 2, 1, 3).reshape(NCORES, P, -1).astype(BF)
    colp4b = colp4.transpose(0, 2, 1, 3).reshape(NCORES, P, -1)
    return dict(N=N, npc=npc, NT=NT, npad=npad, NCHUNK=NCHUNK, E_pad=E_pad,
                chunk_meta=chunk_meta, rowflat=rowrel.astype(BF),
                rowp4b=rowp4b, colp4b=colp4b, ein1=ein1, invP=invP, mskP=mskP,
                x_own=x_own.reshape(NCORES, npc, Fx))


def kernel(x, edge_attr, edge_index, **wts):
    x = np.asarray(x, np.float32)
    edge_attr = np.asarray(edge_attr, np.float32)
    edge_index = np.asarray(edge_index)
    wts = {k: np.asarray(v, np.float32) for k, v in wts.items()}
    import os
    return _run(x, edge_attr, edge_index, wts,
                trace=os.environ.get("BASS_KERNEL_TRACE", "0") == "1")


def _pair_groups(chunks):
    """Group a tile's chunks into pairs of full-width chunks (rem alone)."""
    groups = []
    i = 0
    while i < len(chunks):
        if (i + 1 < len(chunks) and chunks[i][2] == CW
                and chunks[i + 1][2] == CW):
            groups.append([chunks[i], chunks[i + 1]])
            i += 2
        else:
            groups.append([chunks[i]])
            i += 1
    return groups


def _run(x, edge_attr, edge_index, wts, trace=False, build_only=False):
    pre = _host_prep(x, edge_attr, edge_index)
    F = x.shape[1]          # 32
    H = wts["e1_w2"].shape[1]  # 64
    FE = edge_attr.shape[1]  # 32
    npc, NT, NCHUNK, E_pad = pre["npc"], pre["NT"], pre["NCHUNK"], pre["E_pad"]

    # bias folding: e*_b2 folded into downstream first-layer biases (the ea
    # streams are stored pre-bias)
    b_hm1 = (wts["n1a_b1"] + wts["e1_b2"] @ wts["n1a_w1"][F:]).reshape(H, 1)
    b_e2 = (wts["e2_b1"] + wts["e1_b2"] @ wts["e2_w1"][2 * H:]).reshape(H, 1)
    b_hm2 = (wts["n2a_b1"] + wts["e2_b2"] @ wts["n2a_w1"][H:]).reshape(H, 1)

    def bf(a):
        return np.ascontiguousarray(a).astype(BF)

    n1a_wq = np.zeros((P, H), BF)
    n1a_wq[0:H] = bf(wts["n1a_w1"][F:])        # ea part
    n1a_wq[H:H + F] = bf(wts["n1a_w1"][:F])    # xcol part
    n2a_wq = np.zeros((P, H), BF)
    n2a_wq[0:H] = bf(wts["n2a_w1"][H:])        # ea part
    n2a_wq[H:2 * H] = bf(wts["n2a_w1"][:H])    # xcol part

    # ein1 feature order: [xrow; ea; xcol]
    e1_w1r = np.concatenate([wts["e1_w1"][:F], wts["e1_w1"][2 * F:],
                             wts["e1_w1"][F:2 * F]])

    # iotaF4T[p, n*4 + k] = n  (for the [P, 128, 4] selT layout)
    iotaF4T = np.repeat(np.arange(P, dtype=np.float32), 4)[None, :]
    iotaF4T = np.tile(iotaF4T, (P, 1)).astype(BF)

    cb = dict(  # bf16 weight consts
        e1_w1r=bf(e1_w1r), e1_w2=bf(wts["e1_w2"]),
        n1a_wq=n1a_wq, n1a_w2=bf(wts["n1a_w2"]),
        n1b_wq=bf(np.concatenate([wts["n1b_w1"][F:], wts["n1b_w1"][:F]])),
        n1b_w2=bf(wts["n1b_w2"]),
        e2_w1u=bf(wts["e2_w1"][:H]),
        e2_w1e=bf(wts["e2_w1"][2 * H:]),
        e2_w1x=bf(wts["e2_w1"][H:2 * H]),
        e2_w2=bf(wts["e2_w2"]),
        n2a_wq=n2a_wq, n2a_w2=bf(wts["n2a_w2"]),
        n2b_wq=bf(np.concatenate([wts["n2b_w1"][H:], wts["n2b_w1"][:H]])),
        n2b_w2=np.concatenate([bf(wts["n2b_w2"]), np.zeros((H, 1), BF)],
                              axis=1),
        iotaF4T=iotaF4T,
        iotaPb=np.arange(P, dtype=np.float32).reshape(P, 1).astype(BF),
    )
    cf = dict(  # f32 consts
        e1_b1=wts["e1_b1"].reshape(H, 1), b_hm1=b_hm1,
        n1b_b1=wts["n1b_b1"].reshape(H, 1), n1b_b2=wts["n1b_b2"].reshape(H, 1),
        b_e2=b_e2, b_hm2=b_hm2, n2b_b1=wts["n2b_b1"].reshape(H, 1),
        b2bc1=np.tile(wts["n1a_b2"][None, :], (P, 1)).astype(np.float32),
        b2bc2=np.tile(wts["n2a_b2"][None, :], (P, 1)).astype(np.float32),
        identF=np.eye(P, dtype=np.float32),
    )
    n2b_b2_val = float(wts["n2b_b2"].reshape(-1)[0])

    nc = bass.Bass(num_swdge_queues=4)

    dp = {}
    for k, v in cb.items():
        dp[k] = nc.declare_dram_parameter(k, list(v.shape), BF16,
                                          isOutput=False)
    for k, v in cf.items():
        dp[k] = nc.declare_dram_parameter(k, list(v.shape), F32, isOutput=False)

    xo_d = nc.declare_dram_parameter("x_own", [npc, F], F32, isOutput=False)
    ein1_d = nc.declare_dram_parameter("ein1", [2 * F + FE, E_pad], BF16,
                                       isOutput=False)
    rowf_d = nc.declare_dram_parameter("rowflat", [E_pad], BF16, isOutput=False)
    NC4 = max(NCHUNK, 1) * 4
    rowp4_d = nc.declare_dram_parameter("rowp4b", [P, NC4], BF16, isOutput=False)
    colp4_d = nc.declare_dram_parameter("colp4b", [P, NC4], I32, isOutput=False)
    inv_d = nc.declare_dram_parameter("invP", [P, NT], F32, isOutput=False)
    msk_d = nc.declare_dram_parameter("mskP", [P, NT], F32, isOutput=False)
    x2_d = nc.declare_dram_parameter("x2", [npc, 4], F32, isOutput=True)

    cm = pre["chunk_meta"]

    with tile.TileContext(nc) as tc:
        with (
            tc.tile_pool(name="cst", bufs=1) as cst,
            tc.tile_pool(name="sb", bufs=2) as sb,
            tc.tile_pool(name="sb3", bufs=3) as sb3,
            tc.tile_pool(name="ps_tc", bufs=1, space="PSUM") as ps_tc,
            tc.tile_pool(name="ps_h1", bufs=2, space="PSUM") as ps_h1,
            tc.tile_pool(name="ps_ea", bufs=2, space="PSUM") as ps_ea,
            tc.tile_pool(name="ps_hm", bufs=1, space="PSUM") as ps_hm,
            tc.tile_pool(name="ps_me", bufs=1, space="PSUM") as ps_me,
            tc.tile_pool(name="ps_ag", bufs=1, space="PSUM") as ps_ag,
            tc.tile_pool(name="dram", bufs=1, space="DRAM") as dram,
        ):
            ct = {}
            for k, v in {**cb, **cf}.items():
                dt_ = BF16 if v.dtype == BF else F32
                t_ = cst.tile(list(v.shape), dt_, name=f"c_{k}")
                nc.sync.dma_start(out=t_[:], in_=dp[k][:])
                ct[k] = t_
            r4all = cst.tile([P, NC4], BF16, name="c_r4")
            nc.sync.dma_start(out=r4all[:], in_=rowp4_d[:])
            c4all = cst.tile([P, NC4], I32, name="c_c4")
            nc.sync.dma_start(out=c4all[:], in_=colp4_d[:])
            invT = cst.tile([P, NT], F32, name="c_inv")
            nc.sync.dma_start(out=invT[:], in_=inv_d[:])
            mskT = cst.tile([P, NT], F32, name="c_msk")
            nc.sync.dma_start(out=mskT[:], in_=msk_d[:])

            ea1T_d = dram.tile([H, E_pad], BF16, name="ea1T")
            x1own_d = dram.tile([npc, H], F32, name="x1own")
            x1full_d = dram.tile([NCORES * npc, H], F32, name="x1full",
                                 addr_space="Shared")

            iF = ct["identF"]

            def relu_dve(out_ap, in_ap, bias_ap):
                nc.vector.tensor_scalar(
                    out=out_ap, in0=in_ap, scalar1=bias_ap, scalar2=0.0,
                    op0=mybir.AluOpType.add, op1=mybir.AluOpType.max)

            def scatter_tail(W, R, cid, isf, isl, hm_sb, mw2, agg_ps):
                # second m-MLP layer (edge-major, fused transpose) + one-hot
                # scatter-accumulate into the tile's PSUM agg
                me_ps = ps_me.tile([P, 4 * H], F32, tag="me")
                for k in range(R):
                    nc.tensor.matmul(me_ps[:, k * H:(k + 1) * H],
                                     lhsT=hm_sb[:, k * P:(k + 1) * P],
                                     rhs=ct[mw2][:], start=True, stop=True)
                me_sb = sb3.tile([P, 4 * H], BF16, tag="mesb")
                nc.vector.tensor_copy(me_sb[:, :R * H], me_ps[:, :R * H])
                # selT[e, n, k] = (rowrel[e, k] == n); bf16 in/out for the
                # DVE fast path; agg lhsT reads the strided [:, :, k] slice
                selT = sb3.tile([P, P, 4], BF16, tag="selT")
                nc.vector.tensor_tensor(
                    out=selT[:, :, :R],
                    in0=r4all[:, cid * 4:cid * 4 + R]
                        .rearrange("p (a b) -> p a b", a=1)
                        .to_broadcast([P, P, R]),
                    in1=ct["iotaF4T"][:].rearrange("p (a b) -> p a b", b=4)
                        [:, :, :R],
                    op=mybir.AluOpType.is_equal)
                for k in range(R):
                    nc.tensor.matmul(agg_ps[:], lhsT=selT[:, :, k],
                                     rhs=me_sb[:, k * H:(k + 1) * H],
                                     start=(isf and k == 0),
                                     stop=(isl and k == R - 1),
                                     skip_group_check=True)

            def node_tail(t, xw, agg_ps, has_edges, b2bc, nwq, nb1,
                          out_hook, nin):
                # mean + bias-mask + node MLP; nin[H:H+xw] already holds x^T
                agg_sb = sb.tile([P, H], F32, tag="aggsb")
                if has_edges:
                    nc.vector.tensor_scalar(
                        out=agg_sb[:], in0=agg_ps[:], scalar1=invT[:, t:t + 1],
                        scalar2=None, op0=mybir.AluOpType.mult)
                else:
                    nc.vector.memset(agg_sb[:], 0.0)
                b2m = sb.tile([P, H], F32, tag="b2m")
                nc.vector.tensor_scalar(
                    out=b2m[:], in0=ct[b2bc][:], scalar1=mskT[:, t:t + 1],
                    scalar2=None, op0=mybir.AluOpType.mult)
                nc.vector.tensor_tensor(out=agg_sb[:], in0=agg_sb[:],
                                        in1=b2m[:], op=mybir.AluOpType.add)
                aT_ps = ps_tc.tile([H, P], F32, tag="tc")
                nc.tensor.transpose(out=aT_ps[:], in_=agg_sb[:], identity=iF[:])
                nc.scalar.activation(nin[0:H, :], aT_ps[:], ACT.Copy)
                hn_ps = ps_hm.tile([H, P], F32, tag="hm")
                nc.tensor.matmul(hn_ps[:], lhsT=ct[nwq][:H + xw, :],
                                 rhs=nin[:H + xw, :], start=True, stop=True)
                hn = sb.tile([H, P], BF16, tag="hn")
                relu_dve(hn[:], hn_ps[:], ct[nb1][:, :1])
                out_hook(t, hn)

            # ---------------- layer 1 ----------------
            def do_allgather(lo, hi):
                nc.gpsimd.collective_compute(
                    "AllGather", mybir.AluOpType.bypass,
                    replica_groups=[list(range(NCORES))],
                    ins=[x1own_d[lo:hi].opt()],
                    outs=[x1full_d.rearrange("(c n) h -> c n h", c=NCORES)[:, lo:hi]
                          .opt()])

            def out1(t, hn):
                x1T_ps = ps_ea.tile([H, P], F32, tag="ea")
                nc.tensor.matmul(x1T_ps[:], lhsT=ct["n1b_w2"][:], rhs=hn[:],
                                 start=True, stop=True)
                x1T = sb.tile([H, P], F32, tag="x1T")
                nc.scalar.activation(x1T[:], x1T_ps[:], ACT.Relu,
                                     bias=ct["n1b_b2"][:, :1])
                x1_ps = ps_me.tile([P, H], F32, tag="me")
                nc.tensor.transpose(out=x1_ps[:], in_=x1T[:],
                                    identity=iF[:H, :H])
                x1sb = sb.tile([P, H], F32, tag="x1sb")
                nc.vector.tensor_copy(x1sb[:], x1_ps[:])
                nc.sync.dma_start(out=x1own_d[t * P:(t + 1) * P, :], in_=x1sb[:])
                if t == NT - 1:
                    do_allgather(0, npc)

            for t in range(NT):
                chunks = [c for c in cm if c[0] == t]
                x_t = sb.tile([P, F], F32, tag="x_t")
                nc.sync.dma_start(out=x_t[:], in_=xo_d[t * P:(t + 1) * P, :])
                xT_ps = ps_tc.tile([F, P], F32, tag="tc")
                nc.tensor.transpose(out=xT_ps[:], in_=x_t[:], identity=iF[:])
                nin = sb.tile([H + F, P], BF16, tag="nin1")
                nc.scalar.activation(nin[H:H + F, :], xT_ps[:], ACT.Copy)
                agg_ps = ps_ag.tile([P, H], F32, tag="agg")
                for grp in _pair_groups(chunks):
                    gw = sum(c[2] for c in grp)
                    gbase = grp[0][1]
                    rhsF = sb.tile([2 * F + FE, 2 * CW], BF16, tag="rhsF")
                    nc.sync.dma_start(out=rhsF[:, :gw],
                                      in_=ein1_d[:, gbase:gbase + gw])
                    m_in = sb.tile([H + F, 2 * CW], BF16, tag="m_in1")
                    for (tt, ebase, W, isf, isl, cid) in grp:
                        off = ebase - gbase
                        R = W // P
                        h1_ps = ps_h1.tile([H, CW], F32, tag="h1")
                        nc.tensor.matmul(h1_ps[:, :W], lhsT=ct["e1_w1r"][:],
                                         rhs=rhsF[:, off:off + W], start=True,
                                         stop=True)
                        h1r = sb3.tile([H, CW], BF16, tag="h1r")
                        relu_dve(h1r[:, :W], h1_ps[:, :W], ct["e1_b1"][:, :1])
                        ea_ps = ps_ea.tile([H, CW], F32, tag="ea")
                        nc.tensor.matmul(ea_ps[:, :W], lhsT=ct["e1_w2"][:],
                                         rhs=h1r[:, :W], start=True, stop=True)
                        nc.scalar.activation(m_in[0:H, off:off + W],
                                             ea_ps[:, :W], ACT.Copy)
                        nc.vector.tensor_copy(m_in[H:H + F, off:off + W],
                                              rhsF[F + FE:, off:off + W])
                        hm_ps = ps_hm.tile([H, CW], F32, tag="hm")
                        nc.tensor.matmul(hm_ps[:, :W],
                                         lhsT=ct["n1a_wq"][0:H + F, :],
                                         rhs=m_in[:, off:off + W], start=True,
                                         stop=True)
                        hm_sb = sb3.tile([H, CW], BF16, tag="hm_sb")
                        nc.scalar.activation(hm_sb[:, :W], hm_ps[:, :W],
                                             ACT.Relu, bias=ct["b_hm1"][:, :1])
                        scatter_tail(W, R, cid, isf, isl, hm_sb, "n1a_w2",
                                     agg_ps)
                    nc.scalar.dma_start(out=ea1T_d[:, gbase:gbase + gw],
                                        in_=m_in[0:H, :gw])
                node_tail(t, F, agg_ps, bool(chunks), "b2bc1",
                          "n1b_wq", "n1b_b1", out1, nin)

            # ---------------- layer 2 ----------------
            def out2(t, hn):
                x2_ps = ps_me.tile([P, 4], F32, tag="me")
                nc.tensor.matmul(x2_ps[:, :2], lhsT=hn[:],
                                 rhs=ct["n2b_w2"][:], start=True, stop=True)
                x2sb = sb.tile([P, 4], F32, tag="x2sb")
                nc.scalar.activation(x2sb[:, :1], x2_ps[:, :1], ACT.Copy,
                                     bias=n2b_b2_val)
                nc.sync.dma_start(out=x2_d[t * P:(t + 1) * P, :1], in_=x2sb[:, :1])

            for t in range(NT):
                chunks = [c for c in cm if c[0] == t]
                x_t = sb.tile([P, H], F32, tag="x_t")
                nc.sync.dma_start(out=x_t[:], in_=x1own_d[t * P:(t + 1) * P, :])
                xT_ps = ps_tc.tile([H, P], F32, tag="tc")
                nc.tensor.transpose(out=xT_ps[:], in_=x_t[:], identity=iF[:])
                nin = sb.tile([2 * H, P], BF16, tag="nin2")
                nc.scalar.activation(nin[H:2 * H, :], xT_ps[:], ACT.Copy)
                # U^T = x1_tile @ e2_w1[0:H] (the x1[row] part of the edge MLP)
                x1T_sb = sb.tile([H, P], BF16, tag="x1Tsb")
                nc.scalar.activation(x1T_sb[:], xT_ps[:], ACT.Copy)
                ut_ps = ps_h1.tile([P, H], F32, tag="h1")
                nc.tensor.matmul(ut_ps[:], lhsT=x1T_sb[:],
                                 rhs=ct["e2_w1u"][:], start=True, stop=True)
                ut_sb = sb.tile([P, H], BF16, tag="ut")
                nc.vector.tensor_copy(ut_sb[:], ut_ps[:])
                agg_ps = ps_ag.tile([P, H], F32, tag="agg")
                for grp in _pair_groups(chunks):
                    gw = sum(c[2] for c in grp)
                    gbase = grp[0][1]
                    ea1p = sb.tile([H, 2 * CW], BF16, tag="ea1p")
                    nc.sync.dma_start(out=ea1p[:, :gw],
                                      in_=ea1T_d[:, gbase:gbase + gw])
                    rbp = sb.tile([P, 2 * CW], BF16, tag="rb")
                    nc.scalar.dma_start(
                        out=rbp[:, :gw],
                        in_=rowf_d[None, gbase:gbase + gw].to_broadcast([P, gw]))
                    for (tt, ebase, W, isf, isl, cid) in grp:
                        off = ebase - gbase
                        R = W // P
                        selN = sb3.tile([P, CW], BF16, tag="selN")
                        nc.vector.tensor_tensor(
                            out=selN[:, :W],
                            in0=ct["iotaPb"][:, 0:1].to_broadcast([P, W]),
                            in1=rbp[:, off:off + W],
                            op=mybir.AluOpType.is_equal)
                        tc_ps = ps_tc.tile([H, CW], F32, tag="tc")
                        for k in range(R):
                            g = sb3.tile([P, H], F32, tag=f"g{k}")
                            nc.gpsimd.indirect_dma_start(
                                out=g[:], out_offset=None, in_=x1full_d[:],
                                in_offset=bass.IndirectOffsetOnAxis(
                                    ap=c4all[:, cid * 4 + k:cid * 4 + k + 1],
                                    axis=0))
                            nc.tensor.transpose(
                                out=tc_ps[:, k * P:(k + 1) * P],
                                in_=g[:], identity=iF[:])
                        xcolT = sb3.tile([H, CW], BF16, tag="xcolT")
                        nc.scalar.activation(xcolT[:, :W], tc_ps[:, :W],
                                             ACT.Copy)
                        h1_ps = ps_h1.tile([H, CW], F32, tag="h1")
                        i1 = nc.tensor.matmul(h1_ps[:, :W], lhsT=ct["e2_w1e"][:],
                                              rhs=ea1p[:, off:off + W],
                                              start=True, stop=False,
                                              skip_group_check=True)
                        i2 = nc.tensor.matmul(h1_ps[:, :W], lhsT=ct["e2_w1x"][:],
                                              rhs=xcolT[:, :W],
                                              start=False, stop=False,
                                              skip_group_check=True)
                        i3 = nc.tensor.matmul(h1_ps[:, :W], lhsT=ut_sb[:],
                                              rhs=selN[:, :W], start=False,
                                              stop=True, skip_group_check=True)
                        add_dep_helper(i2.ins, i1.ins, sync=False,
                                       reason="h1 accumulate order")
                        add_dep_helper(i3.ins, i2.ins, sync=False,
                                       reason="h1 accumulate order")
                        h1r = sb3.tile([H, CW], BF16, tag="h1r")
                        relu_dve(h1r[:, :W], h1_ps[:, :W], ct["b_e2"][:, :1])
                        ea_ps = ps_ea.tile([H, CW], F32, tag="ea")
                        nc.tensor.matmul(ea_ps[:, :W], lhsT=ct["e2_w2"][:],
                                         rhs=h1r[:, :W], start=True, stop=True)
                        m_in = sb.tile([2 * H, CW], BF16, tag="m_in2")
                        nc.scalar.activation(m_in[0:H, :W], ea_ps[:, :W],
                                             ACT.Copy)
                        nc.vector.tensor_copy(m_in[H:2 * H, :W], xcolT[:, :W])
                        hm_ps = ps_hm.tile([H, CW], F32, tag="hm")
                        nc.tensor.matmul(hm_ps[:, :W], lhsT=ct["n2a_wq"][:],
                                         rhs=m_in[:, :W], start=True, stop=True)
                        hm_sb = sb3.tile([H, CW], BF16, tag="hm_sb")
                        nc.scalar.activation(hm_sb[:, :W], hm_ps[:, :W],
                                             ACT.Relu, bias=ct["b_hm2"][:, :1])
                        scatter_tail(W, R, cid, isf, isl, hm_sb, "n2a_w2",
                                     agg_ps)
                node_tail(t, H, agg_ps, bool(chunks), "b2bc2",
                          "n2b_wq", "n2b_b1", out2, nin)

    if SPLIT_WAITS:
        _split_multi_waits(nc)

    in_maps = []
    for c in range(NCORES):
        m = dict(cb)
        m.update(cf)
        m["x_own"] = pre["x_own"][c]
        m["ein1"] = pre["ein1"][c]
        m["rowflat"] = pre["rowflat"][c]
        m["rowp4b"] = pre["rowp4b"][c]
        m["colp4b"] = pre["colp4b"][c]
        m["invP"] = pre["invP"][c]
        m["mskP"] = pre["mskP"][c]
        in_maps.append(m)

    kernel.last_nc = nc
    kernel.last_in_maps = in_maps
    if build_only:
        return pre
    r = run_bass_kernel_spmd(nc, in_maps, list(range(NCORES)), trace=trace)
    kernel.last_results = r
    out = np.concatenate([r.results[c]["x2"][:, :1] for c in range(NCORES)], axis=0)
    return out[:pre["N"]].astype(np.float32)
